# revision 19
# baseline (speedup 1.0000x reference)
"""Trainium2 Bass kernel for a pre-norm transformer block (nn_Block_38843684225792).

Full inputs -> full outputs. Sharding: data-parallel over batch, one batch
element per NeuronCore (8 cores). Inside each core the block is computed
channel-major (channels on SBUF partitions) so every matmul contracts over
the partition dim without extra transposes; x is transposed once on entry
and the result transposed back on exit via PE transposes.

Shapes (per core): x [1024, 768], heads=12, hd=64, mlp hidden=3072.
"""

import os
import sys

sys.path.insert(0, "/opt/trn_rl_repo")

import numpy as np

import concourse.bass as bass
import concourse.tile as tile
from concourse import bacc, mybir
from concourse.bass_utils import run_bass_kernel_spmd
from concourse.masks import make_identity

F32 = mybir.dt.float32
F32R = mybir.dt.float32r
BF16 = mybir.dt.bfloat16
AF = mybir.ActivationFunctionType

N_CORES = 8
S = 1024          # sequence length per core
C = 768           # model dim
H = 12            # heads
HD = 64           # head dim
HID = 3072        # mlp hidden
NCH = C // 128    # 6 channel chunks
NT = S // 128     # 8 token chunks
NFH = HID // 128  # 24 hidden chunks
EPS = 1e-5
ATT_SCALE = HD ** -0.5  # 0.125

_cached = {}



def _ln_half(nc, ps, work, src_tiles, dst, n, g_ap, b_ap, ones_col, eps_ap,
             tagp=""):
    """One 512-token half of channel-major LayerNorm (see _ln_channel_major)."""
    sl = slice(512 * n, 512 * (n + 1))
    # single [1,1024] sums tile on the (otherwise idle) "po" psum tag so the
    # long-held sums never stall the ps3 streaming ring: sumx in cols 0:512,
    # sumx2 in cols 512:1024.
    psum2 = ps.tile([1, S], F32, tag="po", bufs=1, name=f"ln_sums{tagp}{n}")
    psx = psum2[:, 0:512]
    psx2 = psum2[:, 512:1024]
    for c in range(NCH):
        x2 = work.tile([128, 512], F32R, tag="x2", bufs=2,
                       name=f"ln_x2_{tagp}{n}{c}")
        nc.vector.tensor_mul(out=x2, in0=src_tiles[c].bitcast(F32)[:, sl],
                             in1=src_tiles[c].bitcast(F32)[:, sl])
        nc.tensor.matmul(psx, ones_col, src_tiles[c][:, sl],
                         start=(c == 0), stop=(c == NCH - 1))
        nc.tensor.matmul(psx2, ones_col, x2,
                         start=(c == 0), stop=(c == NCH - 1))
    # stats on [1, 512] rows
    mu = work.tile([1, 512], F32, tag="stats", bufs=2, name=f"ln_mu{tagp}{n}")
    nc.scalar.mul(out=mu, in_=psx, mul=1.0 / C)
    ex2 = work.tile([1, 512], F32, tag="stats", bufs=2, name=f"ln_ex2{tagp}{n}")
    nc.scalar.mul(out=ex2, in_=psx2, mul=1.0 / C)
    var = work.tile([1, 512], F32, tag="stats2", bufs=2, name=f"ln_var{tagp}{n}")
    nc.vector.tensor_mul(out=var, in0=mu, in1=mu)
    nc.vector.tensor_sub(out=var, in0=ex2, in1=var)
    nc.scalar.activation(out=var, in_=var, func=AF.Ln, bias=eps_ap, scale=1.0)
    rstd = work.tile([1, 512], F32, tag="stats2", bufs=2, name=f"ln_rstd{tagp}{n}")
    nc.scalar.activation(out=rstd, in_=var, func=AF.Exp, bias=0.0, scale=-0.5)
    muR = work.tile([1, 512], F32, tag="stats", bufs=2, name=f"ln_muR{tagp}{n}")
    nc.vector.tensor_mul(out=muR, in0=mu, in1=rstd)
    # broadcast across partitions on GpSimd
    b_rstd = work.tile([128, 512], F32, tag="bca", bufs=1, name=f"ln_brstd{tagp}{n}")
    nc.gpsimd.partition_broadcast(b_rstd, rstd)
    b_muR = work.tile([128, 512], F32, tag="bcb", bufs=1, name=f"ln_bmuR{tagp}{n}")
    nc.gpsimd.partition_broadcast(b_muR, muR)
    # apply: h = (x * rstd_bc - muR_bc) * g + b
    for c in range(NCH):
        t = work.tile([128, 512], F32, tag="tmp", bufs=2,
                      name=f"ln_t_{tagp}{n}{c}")
        nc.vector.tensor_mul(out=t, in0=src_tiles[c].bitcast(F32)[:, sl],
                             in1=b_rstd)
        nc.gpsimd.tensor_sub(out=t, in0=t, in1=b_muR)
        nc.scalar.activation(out=dst[c][:, sl], in_=t, func=AF.Identity,
                             bias=b_ap[:, c:c + 1], scale=g_ap[:, c:c + 1])


def build():
    nc = bacc.Bacc(None, target_bir_lowering=False, debug=False)
    x_d = nc.declare_dram_parameter("x", [S, C], F32, isOutput=False)
    ln1_g_d = nc.declare_dram_parameter("ln1_g", [C], F32, isOutput=False)
    ln1_b_d = nc.declare_dram_parameter("ln1_b", [C], F32, isOutput=False)
    w_qkv_d = nc.declare_dram_parameter("w_qkv", [C, 3 * C], F32, isOutput=False)
    w_proj_d = nc.declare_dram_parameter("w_proj", [C, C], F32, isOutput=False)
    b_proj_d = nc.declare_dram_parameter("b_proj", [C], F32, isOutput=False)
    ln2_g_d = nc.declare_dram_parameter("ln2_g", [C], F32, isOutput=False)
    ln2_b_d = nc.declare_dram_parameter("ln2_b", [C], F32, isOutput=False)
    w_fc1_d = nc.declare_dram_parameter("w_fc1", [C, HID], F32, isOutput=False)
    b_fc1_d = nc.declare_dram_parameter("b_fc1", [HID], F32, isOutput=False)
    w_fc2_d = nc.declare_dram_parameter("w_fc2", [HID, C], F32, isOutput=False)
    b_fc2_d = nc.declare_dram_parameter("b_fc2", [C], F32, isOutput=False)
    out_d = nc.declare_dram_parameter("out", [S, C], F32, isOutput=True)

    from contextlib import ExitStack
    with tile.TileContext(nc) as tc, ExitStack() as ctx:
        consts = ctx.enter_context(tc.tile_pool(name="consts", bufs=1))
        arena = ctx.enter_context(tc.tile_pool(name="arena", bufs=1))
        work = ctx.enter_context(tc.tile_pool(name="work", bufs=1))
        ps = ctx.enter_context(tc.tile_pool(name="ps", bufs=2, space="PSUM"))
        dram = ctx.enter_context(tc.tile_pool(name="dram", bufs=1, space="DRAM"))

        # ---------------- constants ----------------
        ident = consts.tile([128, 128], F32, name="ident")
        make_identity(nc, ident)
        ones_f32 = consts.tile([128, 1], F32, name="ones_f32")
        nc.vector.memset(ones_f32, 1.0)
        ones_col = consts.tile([128, 1], F32R, name="ones_col")
        nc.vector.tensor_copy(out=ones_col, in_=ones_f32)
        eps_ap = consts.tile([1, 1], F32, name="eps_ap")
        nc.vector.memset(eps_ap, EPS)

        def load_chanvec(dram_t, name, width):
            t = consts.tile([128, width], F32, name=name)
            nc.sync.dma_start(out=t, in_=dram_t.ap().rearrange("(o p) -> p o", p=128))
            return t

        g1 = load_chanvec(ln1_g_d, "g1", NCH)
        b1 = load_chanvec(ln1_b_d, "b1", NCH)
        g2 = load_chanvec(ln2_g_d, "g2", NCH)
        b2 = load_chanvec(ln2_b_d, "b2", NCH)
        bp = load_chanvec(b_proj_d, "bp", NCH)
        bf1 = load_chanvec(b_fc1_d, "bf1", NFH)
        bf2 = load_chanvec(b_fc2_d, "bf2", NCH)

        # ---------------- stage A + LN1, interleaved per token half ------
        # Token half n needs only transposes of token chunks 4n..4n+3, so the
        # second batch of transposes runs under half 0's serial stats chain.
        xT = [arena.tile([128, S], F32R, tag=f"xT{c}", name=f"xT{c}")
              for c in range(NCH)]
        hT = [arena.tile([128, S], F32R, tag=f"hT{c}", name=f"hT{c}")
              for c in range(NCH)]

        def transpose_in(a):
            x_sb = work.tile([128, C], F32, tag="x_sb", bufs=2, name=f"x_sb{a}")
            nc.sync.dma_start(out=x_sb, in_=x_d.ap()[128 * a:128 * (a + 1), :])
            for c in range(NCH):
                pst = ps.tile([128, 128], F32, tag="ps3", bufs=3,
                              name=f"ptx{a}_{c}")
                nc.tensor.transpose(pst, x_sb[:, 128 * c:128 * (c + 1)], ident)
                nc.vector.tensor_copy(out=xT[c][:, 128 * a:128 * (a + 1)], in_=pst)

        for a in range(4):
            transpose_in(a)
        _ln_half(nc, ps, work, xT, hT, 0, g1, b1, ones_col, eps_ap, tagp="l1")
        for a in range(4, 8):
            transpose_in(a)
        _ln_half(nc, ps, work, xT, hT, 1, g1, b1, ones_col, eps_ap, tagp="l1")

        # ---------------- stage C0: v = h @ w_v  (token-major) -----------
        # v_aug[p, mt, head, 66]: [v(64), one, pad]; the ones column makes
        # the O matmul emit softmax denominators at psum partition 64.
        v_aug = arena.tile([128, NT, H, 66], BF16, tag="v_aug", name="v_aug")
        nc.vector.memset(v_aug[:, :, :, 64:65], 1.0)
        with tc.tile_pool(name="wvp", bufs=1) as wvp:
            wv = []
            for i in range(12):
                n, ko = i // NCH, i % NCH
                w = wvp.tile([128, 384], F32R, tag=f"wv{i}", bufs=1,
                             name=f"wv{n}_{ko}")
                nc.sync.dma_start(
                    out=w,
                    in_=w_qkv_d.ap()[128 * ko:128 * (ko + 1),
                                     1536 + 384 * n:1536 + 384 * (n + 1)]
                    .bitcast(F32R))
                wv.append(w)
            for n in range(2):  # halves of the 768 v-channels (heads 6n..6n+5)
                for mt in range(NT):
                    pv = ps.tile([128, 384], F32, tag="ps3", bufs=3,
                                 name=f"pv{n}_{mt}")
                    for ko in range(NCH):
                        nc.tensor.matmul(
                            pv, hT[ko][:, 128 * mt:128 * (mt + 1)],
                            wv[n * NCH + ko],
                            start=(ko == 0), stop=(ko == NCH - 1))
                    pv3 = pv.rearrange("p (j d) -> p j d", d=HD)
                    nc.vector.tensor_copy(
                        out=v_aug[:, mt, 6 * n:6 * n + 6, 0:64], in_=pv3)

        # ---------------- weight re-layout scratch (DMAs emitted inside the
        # attention pair loop so they queue behind the latency-critical
        # qk/proj weight loads) -------------------------------------------
        w1r = dram.tile([NFH, 128, NCH, 128], F32, name="w1r")
        w2r = dram.tile([NCH, 128, NFH, 128], F32, name="w2r")

        def emit_reorg(i):
            if i < NFH:
                nc.sync.dma_start(
                    out=w1r[i],
                    in_=w_fc1_d.ap()[:, 128 * i:128 * (i + 1)]
                    .rearrange("(ko ki) m -> ki ko m", ki=128))
            else:
                j = i - NFH
                nc.sync.dma_start(
                    out=w2r[j],
                    in_=w_fc2_d.ap()[:, 128 * j:128 * (j + 1)]
                    .rearrange("(ko ki) m -> ki ko m", ki=128))

        # ---------------- stages C1+D: qk^T and attention per head pair --
        attnT = [arena.tile([128, S], F32R, tag=f"attnT{c}", name=f"attnT{c}")
                 for c in range(NCH)]
        wp_tiles = []
        with tc.tile_pool(name="wqkp", bufs=1) as wqkp:
            def emit_qkT(p):
                qkT = []
                for which, m in (("q", p), ("k", 6 + p)):
                    wqk = wqkp.tile([128, NCH, 128], F32R, tag=f"w{which}",
                                    bufs=2, name=f"w{which}{p}")
                    nc.sync.dma_start(
                        out=wqk,
                        in_=w_qkv_d.ap()[:, 128 * m:128 * (m + 1)]
                        .rearrange("(ko ki) m -> ki ko m", ki=128).bitcast(F32R))
                    pqk = ps.tile([128, S], F32, tag="ps3", bufs=3, name=f"pqk{which}{p}")
                    for n in range(2):
                        sl = slice(512 * n, 512 * (n + 1))
                        for ko in range(NCH):
                            nc.tensor.matmul(pqk[:, sl], wqk[:, ko, :],
                                             hT[ko][:, sl],
                                             start=(ko == 0), stop=(ko == NCH - 1))
                    t = arena.tile([128, S], BF16, tag=f"{which}T", bufs=2,
                                   name=f"{which}T{p}")
                    nc.vector.tensor_copy(out=t, in_=pqk)
                    qkT.append(t)
                return qkT

            qkT_next = emit_qkT(0)
            for p in range(6):  # head pair (2p, 2p+1)
                qTp, kTp = qkT_next

                def attend(hh):
                    base = 64 * (hh % 2)
                    po = ps.tile([128, S], F32, tag="po", bufs=1, name=f"po{hh}")
                    for kc in range(NT):
                        kcs = slice(128 * kc, 128 * (kc + 1))
                        pS = ps.tile([128, S], F32, tag="ps3", bufs=3,
                                     name=f"pS{hh}_{kc}")
                        for n in range(2):
                            sl = slice(512 * n, 512 * (n + 1))
                            nc.tensor.matmul(pS[:, sl],
                                             kTp[base:base + 64, kcs],
                                             qTp[base:base + 64, sl])
                        expS = work.tile([128, S], BF16, tag="expS", bufs=3,
                                         name=f"expS{hh}_{kc}")
                        nc.scalar.activation(out=expS, in_=pS, func=AF.Exp,
                                             bias=0.0, scale=ATT_SCALE)
                        for n in range(2):
                            sl = slice(512 * n, 512 * (n + 1))
                            nc.tensor.matmul(
                                po[0:65, sl], v_aug[:, kc, hh, 0:65], expS[:, sl],
                                start=(kc == 0), stop=(kc == NT - 1))
                    return po

                def normalize(hh, po):
                    # normalize rows by the ones-column row sums (all off-PE)
                    o_sb = work.tile([65, S], F32R, tag="o_sb", bufs=2,
                                     name=f"o_sb{hh}")
                    nc.vector.tensor_copy(out=o_sb, in_=po[0:65, :])
                    r_raw = work.tile([1, S], F32, tag="r_raw", bufs=2,
                                      name=f"r_raw{hh}")
                    nc.sync.dma_start(out=r_raw, in_=o_sb[64:65, :].bitcast(F32))
                    r_rec = work.tile([1, S], F32, tag="r_rec", bufs=2,
                                      name=f"r_rec{hh}")
                    nc.vector.reciprocal_approx_fast(out=r_rec, in_=r_raw)
                    pr_sb = work.tile([128, S], F32, tag="pr", bufs=1,
                                      name=f"pr{hh}")
                    nc.gpsimd.partition_broadcast(pr_sb, r_rec)
                    c2 = hh // 2
                    if hh % 2 == 0:
                        nc.vector.tensor_mul(out=attnT[c2][0:64, :],
                                             in0=o_sb[0:64, :].bitcast(F32),
                                             in1=pr_sb[0:64, :])
                    else:
                        stg = work.tile([64, S], F32R, tag="stg", bufs=1,
                                        name=f"stg{hh}")
                        nc.vector.tensor_mul(out=stg,
                                             in0=o_sb[0:64, :].bitcast(F32),
                                             in1=pr_sb[0:64, :])
                        nc.sync.dma_start(out=attnT[c2][64:128, :], in_=stg)

                po0 = attend(2 * p)
                normalize(2 * p, po0)
                po1 = attend(2 * p + 1)
                # next pair's qk^T is emitted BEFORE the second normalize so
                # its PE matmuls and DVE casts aren't queued behind it.
                if p < 5:
                    qkT_next = emit_qkT(p + 1)
                if p == 4:
                    # prefetch proj weights during the last attention pair
                    for mc in range(NCH):
                        wp = wqkp.tile([128, NCH, 128], F32R, tag="wp", bufs=2,
                                       name=f"wp{mc}")
                        nc.sync.dma_start(
                            out=wp,
                            in_=w_proj_d.ap()[:, 128 * mc:128 * (mc + 1)]
                            .rearrange("(ko ki) m -> ki ko m", ki=128)
                            .bitcast(F32R))
                        wp_tiles.append(wp)
                for i in range(5 * p, 5 * p + 5):
                    emit_reorg(i)
                if p == 5:
                    for i in range(25, 30):
                        emit_reorg(i)
                normalize(2 * p + 1, po1)

        # ---------------- stage E: proj + residual -----------------------
        out1T = []
        for mc in range(NCH):
            wp = wp_tiles[mc]
            py = ps.tile([128, S], F32, tag="ps3", bufs=3, name=f"py{mc}")
            for n in range(2):
                sl = slice(512 * n, 512 * (n + 1))
                for ko in range(NCH):
                    nc.tensor.matmul(py[:, sl], wp[:, ko, :], attnT[ko][:, sl],
                                     start=(ko == 0), stop=(ko == NCH - 1))
            t = work.tile([128, S], F32, tag="tmp_e", bufs=2, name=f"et{mc}")
            nc.vector.tensor_add(out=t, in0=py, in1=xT[mc].bitcast(F32))
            o1 = arena.tile([128, S], F32R, tag=f"out1T{mc}", name=f"out1T{mc}")
            nc.scalar.activation(out=o1, in_=t, func=AF.Identity,
                                 bias=bp[:, mc:mc + 1], scale=1.0)
            out1T.append(o1)

        # ---------------- stage F: LN2 -----------------------------------
        h2T = [arena.tile([128, S], F32R, tag=f"hT{c}", name=f"h2T{c}")
               for c in range(NCH)]
        for n in range(2):
            _ln_half(nc, ps, work, out1T, h2T, n, g2, b2, ones_col, eps_ap,
                     tagp="l2")

        # ---------------- stages G+H: MLP (bf16, full token width) ------
        with tc.tile_pool(name="wmlp", bufs=1) as wmlp:
            # fc1 + gelu: a1 tile j ([128, 2048] bf16) holds hidden chunks
            # (2j cols 0:1024, 2j+1 cols 1024:2048).
            a1 = []
            for j in range(12):
                tag = f"xT{j}" if j < 6 else f"attnT{j - 6}"
                a1.append(arena.tile([128, 2 * S], BF16, tag=tag,
                                     name=f"a1_{j}"))
            for mc in range(NFH):
                w1 = wmlp.tile([128, NCH, 128], F32R, tag="w1", bufs=2,
                               name=f"w1_{mc}")
                nc.sync.dma_start(out=w1, in_=w1r[mc].bitcast(F32R))
                pg = ps.tile([128, S], F32, tag="ps3", bufs=3, name=f"pg{mc}")
                for n in range(2):
                    sl = slice(512 * n, 512 * (n + 1))
                    for ko in range(NCH):
                        nc.tensor.matmul(pg[:, sl], w1[:, ko, :],
                                         h2T[ko][:, sl],
                                         start=(ko == 0), stop=(ko == NCH - 1))
                dst = a1[mc // 2][:, S * (mc % 2):S * (mc % 2) + S]
                nc.scalar.activation(out=dst, in_=pg, func=AF.Gelu,
                                     bias=bf1[:, mc:mc + 1], scale=1.0)
            # fc2 + bias + residual into out1T
            for mc in range(NCH):
                w2fa = wmlp.tile([128, NFH // 2, 128], F32, tag="w2f",
                                 bufs=1, name=f"w2fa_{mc}")
                nc.sync.dma_start(out=w2fa, in_=w2r[mc][:, 0:12, :])
                w2a = wmlp.tile([128, NFH // 2, 128], BF16, tag="w2",
                                bufs=2, name=f"w2a_{mc}")
                nc.vector.tensor_copy(out=w2a, in_=w2fa)
                w2fb = wmlp.tile([128, NFH // 2, 128], F32, tag="w2f",
                                 bufs=1, name=f"w2fb_{mc}")
                nc.sync.dma_start(out=w2fb, in_=w2r[mc][:, 12:24, :])
                w2b = wmlp.tile([128, NFH // 2, 128], BF16, tag="w2",
                                bufs=2, name=f"w2b_{mc}")
                nc.vector.tensor_copy(out=w2b, in_=w2fb)
                py2 = ps.tile([128, S], F32, tag="ps3", bufs=3, name=f"py2_{mc}")
                for n in range(2):
                    sl = slice(512 * n, 512 * (n + 1))
                    for f in range(NFH):
                        wt = w2a if f < 12 else w2b
                        rhs = a1[f // 2][:, S * (f % 2) + 512 * n:
                                         S * (f % 2) + 512 * (n + 1)]
                        nc.tensor.matmul(py2[:, sl], wt[:, f % 12, :], rhs,
                                         start=(f == 0), stop=(f == NFH - 1))
                t = work.tile([128, S], F32, tag="tmp_e", bufs=2,
                              name=f"ht{mc}")
                nc.scalar.activation(out=t, in_=py2, func=AF.Identity,
                                     bias=bf2[:, mc:mc + 1], scale=1.0)
                nc.vector.tensor_add(out=out1T[mc],
                                     in0=out1T[mc].bitcast(F32), in1=t)

        # ---------------- stage I: transpose back + store ----------------
        for a in range(NT):
            o_out = work.tile([128, C], F32, tag="x_sb", bufs=2, name=f"o_out{a}")
            for mc in range(NCH):
                pst = ps.tile([128, 128], F32, tag="ps3", bufs=3, name=f"pto{a}_{mc}")
                nc.tensor.transpose(
                    pst, out1T[mc].bitcast(F32)[:, 128 * a:128 * (a + 1)], ident)
                nc.vector.tensor_copy(out=o_out[:, 128 * mc:128 * (mc + 1)],
                                      in_=pst)
            nc.sync.dma_start(out=out_d.ap()[128 * a:128 * (a + 1), :], in_=o_out)

    nc.compile()
    return nc


def _get_nc():
    if "nc" not in _cached:
        _cached["nc"] = build()
    return _cached["nc"]


def kernel(**inputs):
    nc = _get_nc()
    x = np.ascontiguousarray(np.asarray(inputs["x"], dtype=np.float32))
    weights = {
        k: np.ascontiguousarray(np.asarray(inputs[k], dtype=np.float32))
        for k in ("ln1_g", "ln1_b", "w_qkv", "w_proj", "b_proj",
                  "ln2_g", "ln2_b", "w_fc1", "b_fc1", "w_fc2", "b_fc2")
    }
    in_maps = [{"x": x[i], **weights} for i in range(N_CORES)]
    trace = bool(int(os.environ.get("BASS_KERNEL_TRACE", "0")))
    res = run_bass_kernel_spmd(nc, in_maps, list(range(N_CORES)), trace=trace)
    _cached["last_exec_time_ns"] = res.exec_time_ns
    out = np.stack([res.results[i]["out"] for i in range(N_CORES)], axis=0)
    return out.astype(np.float32)
